# revision 5
# baseline (speedup 1.0000x reference)
# Bidirectional multi-head attention (key-padding mask) on 8 Trainium2 cores.
#
# Sharding: core = (batch b, head-group hg); B=4 x 2 head-groups of 8 heads.
# Each core computes y_partial^T [C, T] for its 8 heads of batch b; host sums
# the two head-group partials per batch and transposes back to [T, C].
#
# Masked keys are gathered away on the host (exactly equivalent to the -inf
# mask), padded to a multiple of 128 with bias -30000 (exp underflows to 0).
#
# Schedule: software-pipelined over head pairs. The scores+exp stream for
# pair p is the ACT-paced backbone; all other PE work (QKV projections for
# pair p+1, V projection, attn@V for pair p-1/p, output projection) is
# interleaved into the PE instruction stream as "filler" groups so the PE
# never idles while ACT chews through the exp tiles. Score matmuls for the
# two heads of a pair sit on partitions 0-63 / 64-127, so adjacent score
# matmuls target disjoint PE row groups and overlap on hardware.
#
# Device layouts (partition dim first):
#   KT(p)   [128, TK]   k^T for pair p (head 2p on parts 0-63, 2p+1 on 64-127)
#   QT(p)   [128, T]    q^T likewise
#   V_sb    [128, KTT, 8, 65] bf16: V rows (key on partitions) + ones col
#   S^T     = matmul(lhsT=KT slice [64,128], rhs=QT [64,512]) -> [128 keys, q]
#   exp     fused on ScalarE: exp(s/8 + bias_k), bias per key partition
#   attn@V  = matmul(lhsT=V_aug [128,65], rhs=attnT [128,512]) accum over kt
#             row 64 of the psum = per-query row-sum (ones column trick)
#   norm    reciprocal(rowsum), DRAM-roundtrip broadcast to 64 partitions,
#             then DVE multiply into outT
#   proj    y^T = matmul(lhsT=WpT strip, rhs=outT) accum over 4 pairs

import sys

import ml_dtypes
import numpy as np

try:
    import concourse.bacc as bacc  # noqa: F401
except ImportError:
    sys.path.insert(0, "/opt/trn_rl_repo")

import concourse.bacc as bacc
import concourse.bass as bass
import concourse.mybir as mybir
import concourse.tile as tile
from concourse.bass_interp import get_hw_module
from concourse.bass_utils import run_bass_kernel_spmd

F32 = mybir.dt.float32
BF16 = mybir.dt.bfloat16
P = 128

D_MODEL = 1024
N_HEADS = 16
HEAD_DIM = 64
B = 4
T_FULL = 2048
HL = 8  # heads per core
PAIRS = HL // 2
CT = D_MODEL // P  # c tiles


def _chunks(total, size):
    out = []
    s = 0
    while s < total:
        out.append((s, min(size, total - s)))
        s += size
    return out


def build_program(T=T_FULL, TK=1152, rounds=1):
    """Build the per-core Bass program. Same program runs on all 8 cores."""
    assert T % 1024 == 0 and TK % P == 0
    KTT = TK // P  # key tiles
    D = HEAD_DIM
    HT = T // 2  # half of queries (one hf)
    NQC = T // 512  # q chunks
    KC = _chunks(TK, 512)  # xk chunks
    QC = _chunks(T, 512)

    nc = bacc.Bacc("TRN2", target_bir_lowering=False, debug=False, num_devices=1)

    xT = nc.dram_tensor("xT", [D_MODEL, T], BF16, kind="ExternalInput")
    xkT = nc.dram_tensor("xkT", [D_MODEL, TK], BF16, kind="ExternalInput")
    WqT = nc.dram_tensor("WqT", [D_MODEL, HL * D], BF16, kind="ExternalInput")
    WkT = nc.dram_tensor("WkT", [D_MODEL, HL * D], BF16, kind="ExternalInput")
    WvT = nc.dram_tensor("WvT", [D_MODEL, HL * D], BF16, kind="ExternalInput")
    WpT = nc.dram_tensor("WpT", [HL * D, D_MODEL], BF16, kind="ExternalInput")
    kbias = nc.dram_tensor("kbias", [P, KTT], F32, kind="ExternalInput")
    yT = nc.dram_tensor("yT", [D_MODEL, T], BF16, kind="ExternalOutput")

    xT_r = xT.ap().rearrange("(ct p) t -> p ct t", p=P)
    xkT_r = xkT.ap().rearrange("(ct p) t -> p ct t", p=P)
    WqT_r = WqT.ap().rearrange("(ct p) o -> p ct o", p=P)
    WkT_r = WkT.ap().rearrange("(ct p) o -> p ct o", p=P)
    WvT_r = WvT.ap().rearrange("(ct p) o -> p ct o", p=P)
    WpT_r = WpT.ap().rearrange("(ct p) o -> p ct o", p=P)
    yT_r = yT.ap().rearrange("(mt p) t -> p mt t", p=P)

    with tile.TileContext(nc) as tc:
        for _round in range(rounds):
            with (
                tc.tile_pool(name="pers", bufs=1) as pers,
                tc.tile_pool(name="ktp", bufs=2) as ktp,
                tc.tile_pool(name="qtp", bufs=2) as qtp,
                tc.tile_pool(name="attp", bufs=1) as attp,
                tc.tile_pool(name="avsp", bufs=2) as avsp,
                tc.tile_pool(name="xs", bufs=2) as xs,
                tc.tile_pool(name="nrm", bufs=2) as nrm,
                tc.tile_pool(name="drp", bufs=2, space="DRAM") as dp,
                tc.tile_pool(name="ys", bufs=2) as yp,
                tc.tile_pool(name="ps_st", bufs=2, space="PSUM") as ps_st,
                tc.tile_pool(name="ps_av", bufs=1, space="PSUM") as ps_av,
                tc.tile_pool(name="ps_dn", bufs=3, space="PSUM") as ps_dn,
            ):
                # ---------------- persistent tiles + prologue DMAs ----------
                V_sb = pers.tile([P, KTT, HL, D + 1], BF16, tag="V")
                outT_sb = pers.tile([P, PAIRS, T], BF16, tag="outT")
                kbias_sb = pers.tile([P, KTT], F32, tag="kbias")
                Wq_sb = pers.tile([P, CT, HL * D], BF16, tag="Wq")
                Wk_sb = pers.tile([P, CT, HL * D], BF16, tag="Wk")
                Wv_sb = pers.tile([P, CT, HL * D], BF16, tag="Wv")
                wp_sb = pers.tile([P, PAIRS, D_MODEL], BF16, tag="wp")

                nc.sync.dma_start(kbias_sb[:], kbias.ap())
                nc.sync.dma_start(Wk_sb[:], WkT_r)
                nc.sync.dma_start(Wq_sb[:], WqT_r)
                nc.sync.dma_start(Wv_sb[:], WvT_r)
                nc.gpsimd.memset(V_sb[:, :, :, D : D + 1], 1.0)

                # runtime-shared handles (set when filler closures execute)
                cur = {}  # ("k"/"q", chunk) -> streamed x tile
                KT_t = {}
                QT_t = {}
                att_t = {}  # pair -> [hh][hf] tiles
                avs_t = {}  # (pair, hh) -> [65, T] tile

                # ---------------- filler group factories --------------------
                def f_xk(c):
                    s, w = KC[c]

                    def go():
                        t = xs.tile([P, CT, 512], BF16, tag="xc", name="xc")
                        nc.sync.dma_start(t[:, :, :w], xkT_r[:, :, s : s + w])
                        cur[("k", c)] = t

                    return go

                def f_xq(c):
                    s, w = QC[c]

                    def go():
                        t = xs.tile([P, CT, 512], BF16, tag="xc", name="xc")
                        nc.sync.dma_start(t[:, :, :w], xT_r[:, :, s : s + w])
                        cur[("q", c)] = t

                    return go

                def f_kproj(pair, c):
                    s, w = KC[c]

                    def go():
                        if pair not in KT_t:
                            KT_t[pair] = ktp.tile(
                                [P, TK], BF16, tag="kt", name=f"kt{pair}"
                            )
                        xc = cur[("k", c)]
                        ps = ps_dn.tile([P, 512], F32, tag="dn")
                        for ct in range(CT):
                            nc.tensor.matmul(
                                ps[:, :w],
                                lhsT=Wk_sb[:, ct, pair * P : (pair + 1) * P],
                                rhs=xc[:, ct, :w],
                                start=(ct == 0),
                                stop=(ct == CT - 1),
                            )
                        nc.vector.tensor_copy(
                            out=KT_t[pair][:, s : s + w], in_=ps[:, :w]
                        )

                    return go

                def f_qproj(pair, c):
                    s, w = QC[c]

                    def go():
                        if pair not in QT_t:
                            QT_t[pair] = qtp.tile(
                                [P, T], BF16, tag="qt", name=f"qt{pair}"
                            )
                        xc = cur[("q", c)]
                        ps = ps_dn.tile([P, 512], F32, tag="dn")
                        for ct in range(CT):
                            nc.tensor.matmul(
                                ps[:, :w],
                                lhsT=Wq_sb[:, ct, pair * P : (pair + 1) * P],
                                rhs=xc[:, ct, :w],
                                start=(ct == 0),
                                stop=(ct == CT - 1),
                            )
                        nc.vector.tensor_copy(
                            out=QT_t[pair][:, s : s + w], in_=ps[:, :w]
                        )

                    return go

                def f_vproj(c, tl):
                    s, _w = KC[c]
                    tt = s // P + tl

                    def go():
                        xc = cur[("k", c)]
                        ps = ps_dn.tile([P, 512], F32, tag="dn")
                        for ct in range(CT):
                            nc.tensor.matmul(
                                ps[:],
                                lhsT=xc[:, ct, tl * P : (tl + 1) * P],
                                rhs=Wv_sb[:, ct, :],
                                start=(ct == 0),
                                stop=(ct == CT - 1),
                            )
                        nc.vector.tensor_copy(
                            out=V_sb[:, tt, :, 0:D],
                            in_=ps[:].rearrange("p (h d) -> p h d", h=HL),
                        )

                    return go

                def f_av(pair, hh, c):
                    s = c * 512
                    hf, so = (0, s) if s < HT else (1, s - HT)

                    def go():
                        if (pair, hh) not in avs_t:
                            avs_t[(pair, hh)] = avsp.tile(
                                [D + 1, T], BF16, tag=f"avs{hh}", name=f"avs{hh}"
                            )
                        h = 2 * pair + hh
                        at = att_t[pair][hh][hf]
                        av = ps_av.tile([P, 512], F32, tag="av")
                        for kt in range(KTT):
                            nc.tensor.matmul(
                                av[0 : D + 1, :],
                                lhsT=V_sb[:, kt, h, :],
                                rhs=at[:, kt, so : so + 512],
                                start=(kt == 0),
                                stop=(kt == KTT - 1),
                            )
                        nc.vector.tensor_copy(
                            out=avs_t[(pair, hh)][:, s : s + 512], in_=av[0 : D + 1, :]
                        )

                    return go

                def f_norm(pair, hh, hf):
                    def go():
                        s = hf * HT
                        a = avs_t[(pair, hh)]
                        rc = nrm.tile([1, HT], BF16, tag="rc")
                        with nc.allow_low_precision(reason="bf16 softmax normalizer"):
                            nc.vector.reciprocal(rc[:], a[D : D + 1, s : s + HT])
                        rcd = dp.tile([1, HT], BF16, tag="rcd")
                        nc.sync.dma_start(rcd[:], rc[:])
                        rcb = nrm.tile([D, HT], BF16, tag="rcb")
                        nc.sync.dma_start(rcb[:], rcd[0:1, :].to_broadcast((D, HT)))
                        nc.vector.tensor_mul(
                            out=outT_sb[hh * 64 : (hh + 1) * 64, pair, s : s + HT],
                            in0=a[0:D, s : s + HT],
                            in1=rcb[:],
                        )

                    return go

                def f_wpdma():
                    def go():
                        nc.sync.dma_start(wp_sb[:], WpT_r)

                    return go

                ysb_t = {}

                def f_proj(m, c, pstags):
                    s = c * 512

                    def go():
                        if c not in ysb_t:
                            ysb_t[c] = yp.tile(
                                [P, CT, 512], BF16, tag="ysb", name="ysb"
                            )
                        ptag = pstags[m % len(pstags)]
                        pool_ = ps_st if ptag == "st" else ps_dn
                        shape = [P, HT] if ptag == "st" else [P, 512]
                        ps = pool_.tile(shape, F32, tag=ptag)
                        for ctp in range(PAIRS):
                            nc.tensor.matmul(
                                ps[:, :512],
                                lhsT=wp_sb[:, ctp, m * P : (m + 1) * P],
                                rhs=outT_sb[:, ctp, s : s + 512],
                                start=(ctp == 0),
                                stop=(ctp == PAIRS - 1),
                            )
                        nc.vector.tensor_copy(out=ysb_t[c][:, m, :], in_=ps[:, :512])

                    return go

                def f_ydma(c):
                    s = c * 512

                    def go():
                        nc.sync.dma_start(yT_r[:, :, s : s + 512], ysb_t[c][:])
                        del ysb_t[c]

                    return go

                # ---------------- score/exp backbone ------------------------
                def emit_half_sts(pair, hf, fillers):
                    """Emit the 2*KTT score tiles of (pair, hf), draining
                    `fillers` evenly across the tiles."""
                    ntiles = 2 * KTT
                    done = 0
                    i = 0
                    for kt in range(KTT):
                        for hh in (0, 1):
                            base = hh * 64
                            pst = ps_st.tile([P, HT], F32, tag="st")
                            for s2 in (0, 512):
                                nc.tensor.matmul(
                                    pst[:, s2 : s2 + 512],
                                    lhsT=KT_t[pair][
                                        base : base + 64, kt * P : (kt + 1) * P
                                    ],
                                    rhs=QT_t[pair][
                                        base : base + 64,
                                        hf * HT + s2 : hf * HT + s2 + 512,
                                    ],
                                    start=True,
                                    stop=True,
                                )
                            nc.scalar.activation(
                                att_t[pair][hh][hf][:, kt, :],
                                pst[:],
                                mybir.ActivationFunctionType.Exp,
                                bias=kbias_sb[:, kt : kt + 1],
                                scale=0.125,
                            )
                            i += 1
                            want = (i * len(fillers)) // ntiles
                            while done < want:
                                fillers[done]()
                                done += 1
                    while done < len(fillers):
                        fillers[done]()
                        done += 1

                def alloc_att(pair):
                    att_t[pair] = [
                        [
                            attp.tile(
                                [P, KTT, HT],
                                BF16,
                                tag=f"at{hh}{hf}",
                                name=f"at{hh}{hf}",
                            )
                            for hf in (0, 1)
                        ]
                        for hh in (0, 1)
                    ]

                # ---------------- ramp --------------------------------------
                for c in range(len(KC)):
                    f_xk(c)()
                    f_kproj(0, c)()
                for c in (0, 1):
                    f_xq(c)()
                    f_qproj(0, c)()

                # ---------------- pair-pipelined windows --------------------
                for pair in range(PAIRS):
                    alloc_att(pair)
                    fa = []  # fillers during hf=0 tiles
                    fb = []  # fillers during hf=1 tiles

                    if pair == 0:
                        # rest of pair0's Q; V proj; pair1 K/Q
                        for c in (2, 3):
                            fa.append(f_xq(c))
                            fa.append(f_qproj(0, c))
                        for c in range(len(KC)):
                            fa.append(f_xk(c))
                            fa.append(f_kproj(1, c))
                            for tl in range(KC[c][1] // P):
                                fa.append(f_vproj(c, tl))
                        for c in range(NQC):
                            fa.append(f_xq(c))
                            fa.append(f_qproj(1, c))
                    else:
                        # AV + norm for (pair-1)'s hf=1 half
                        for c in (NQC // 2, NQC // 2 + 1):
                            fa.append(f_av(pair - 1, 0, c))
                            fa.append(f_av(pair - 1, 1, c))
                        fa.append(f_norm(pair - 1, 0, 1))
                        fa.append(f_norm(pair - 1, 1, 1))
                        if pair < PAIRS - 1:
                            # K/Q proj for pair+1
                            for c in range(len(KC)):
                                fa.append(f_xk(c))
                                fa.append(f_kproj(pair + 1, c))
                            for c in range(NQC):
                                fa.append(f_xq(c))
                                fa.append(f_qproj(pair + 1, c))
                        else:
                            fa.append(f_wpdma())

                    # this pair's hf=0 AV + norm (emitted strictly after all
                    # hf=0 score tiles -> no PE-stream deadlock)
                    for c in (0, 1):
                        fb.append(f_av(pair, 0, c))
                        fb.append(f_av(pair, 1, c))
                    fb.append(f_norm(pair, 0, 0))
                    fb.append(f_norm(pair, 1, 0))
                    if pair == PAIRS - 1:
                        # proj for the first half's chunks
                        for c in (0, 1):
                            for m in range(CT):
                                fb.append(f_proj(m, c, ("dn",)))
                            fb.append(f_ydma(c))

                    emit_half_sts(pair, 0, fa)
                    emit_half_sts(pair, 1, fb)

                # ---------------- tail --------------------------------------
                last = PAIRS - 1
                for c in (NQC // 2, NQC // 2 + 1):
                    f_av(last, 0, c)()
                    f_av(last, 1, c)()
                f_norm(last, 0, 1)()
                f_norm(last, 1, 1)()
                for c in (2, 3):
                    for m in range(CT):
                        f_proj(m, c, ("dn", "st", "st"))()
                    f_ydma(c)()

    return nc


def prep_core_inputs(x, pad_mask, W_qkv, W_proj, b, hg, TK):
    """Host-side shard prep for core (b, hg)."""
    C = D_MODEL
    D = HEAD_DIM
    xb = np.asarray(x[b], dtype=np.float32)  # [T, C]
    mask = np.asarray(pad_mask[b])
    idx = np.nonzero(~mask)[0]
    cnt = len(idx)
    assert cnt <= TK, f"key count {cnt} exceeds TK={TK}"

    BF = ml_dtypes.bfloat16
    xT = np.ascontiguousarray(xb.T).astype(BF)  # [C, T]
    xkT = np.zeros((C, TK), dtype=BF)
    xkT[:, :cnt] = xb[idx].T.astype(BF)

    kb = np.zeros((TK,), dtype=np.float32)
    kb[cnt:] = -30000.0
    kbias = np.ascontiguousarray(kb.reshape(TK // P, P).T)  # [128, KTT]

    Wq = W_qkv[0:C].reshape(N_HEADS, D, C)
    Wk = W_qkv[C : 2 * C].reshape(N_HEADS, D, C)
    Wv = W_qkv[2 * C : 3 * C].reshape(N_HEADS, D, C)
    heads = range(hg * HL, (hg + 1) * HL)
    WqT = np.ascontiguousarray(np.concatenate([Wq[h] for h in heads], axis=0).T)
    WkT = np.ascontiguousarray(np.concatenate([Wk[h] for h in heads], axis=0).T)
    WvT = np.ascontiguousarray(np.concatenate([Wv[h] for h in heads], axis=0).T)
    WpT = np.ascontiguousarray(
        np.concatenate([W_proj[:, h * D : (h + 1) * D] for h in heads], axis=1).T
    )
    return {
        "xT": xT,
        "xkT": xkT,
        "WqT": WqT.astype(BF),
        "WkT": WkT.astype(BF),
        "WvT": WvT.astype(BF),
        "WpT": WpT.astype(BF),
        "kbias": kbias,
    }


def kernel(x, pad_mask, W_qkv, W_proj):
    x = np.asarray(x, dtype=np.float32)
    pad_mask = np.asarray(pad_mask, dtype=bool)
    W_qkv = np.asarray(W_qkv, dtype=np.float32)
    W_proj = np.asarray(W_proj, dtype=np.float32)
    Bv, T, C = x.shape

    counts = (~pad_mask).sum(axis=1)
    TK = max(int(-(-counts.max() // P)) * P, P)

    nc = build_program(T=T, TK=TK)
    nc.compile()
    nc.m = get_hw_module(nc.m)

    in_maps = []
    for c in range(8):
        b, hg = c // 2, c % 2
        in_maps.append(prep_core_inputs(x, pad_mask, W_qkv, W_proj, b, hg, TK))

    res = None
    for attempt in range(3):
        try:
            res = run_bass_kernel_spmd(nc, in_maps, core_ids=list(range(8)))
            break
        except Exception:
            if attempt == 2:
                raise
            import time as _time

            _time.sleep(5.0)

    y = np.empty((Bv, T, C), dtype=np.float32)
    for b in range(Bv):
        yT = res.results[2 * b]["yT"].astype(np.float32) + res.results[2 * b + 1][
            "yT"
        ].astype(np.float32)
        y[b] = yT.T
    return y


# revision 12
# speedup vs baseline: 1.0056x; 1.0056x over previous
# Bidirectional multi-head attention (key-padding mask) on 8 Trainium2 cores.
#
# Sharding: core = (batch b, head-group hg); B=4 x 2 head-groups of 8 heads.
# Each core computes y_partial^T [C, T] for its 8 heads of batch b; host sums
# the two head-group partials per batch and transposes back to [T, C].
#
# Masked keys are gathered away on the host (exactly equivalent to the -inf
# mask), padded to a multiple of 128 with bias -30000 (exp underflows to 0).
#
# Schedule: software-pipelined over head pairs (and across timing rounds).
# The scores+exp stream is the ACT-paced backbone; all other PE work (QKV
# projections for the next pair, V projection, attn@V for the previous pair,
# output projection, next round's ramp) is interleaved into the PE stream as
# micro-fillers (~400-900ns of PE work each) so the PE neither starves the
# ACT engine nor idles. Score matmuls for the two heads of a pair sit on
# partitions 0-63 / 64-127, so adjacent score matmuls target disjoint PE row
# groups and overlap on hardware.
#
# Device layouts (partition dim first):
#   KT(p)   [128, TK]   k^T for pair p (head 2p on parts 0-63, 2p+1 on 64-127)
#   QT(p)   [128, T]    q^T likewise
#   V_sb    [128, KTT, 8, 65] bf16: V rows (key on partitions) + ones col
#   S^T     = matmul(lhsT=KT slice [64,128], rhs=QT [64,512]) -> [128 keys, q]
#   exp     fused on ScalarE: exp(s/8 + bias_k), bias per key partition
#   attn@V  = matmul(lhsT=V_aug [128,65], rhs=attnT [128,512]) accum over kt
#             row 64 of the psum = per-query row-sum (ones column trick)
#   norm    reciprocal(rowsum), DRAM-roundtrip broadcast to 64 partitions,
#             then DVE multiply into outT
#   proj    y^T = matmul(lhsT=WpT strip, rhs=outT) accum over 4 pairs

import sys

import ml_dtypes
import numpy as np

try:
    import concourse.bacc as bacc  # noqa: F401
except ImportError:
    sys.path.insert(0, "/opt/trn_rl_repo")

import concourse.bacc as bacc
import concourse.bass as bass
import concourse.mybir as mybir
import concourse.tile as tile
from concourse.bass_interp import get_hw_module
from concourse.bass_utils import run_bass_kernel_spmd

F32 = mybir.dt.float32
BF16 = mybir.dt.bfloat16
P = 128

D_MODEL = 1024
N_HEADS = 16
HEAD_DIM = 64
B = 4
T_FULL = 2048
HL = 8  # heads per core
PAIRS = HL // 2
CT = D_MODEL // P  # c tiles


def _chunks(total, size):
    out = []
    s = 0
    while s < total:
        out.append((s, min(size, total - s)))
        s += size
    return out


def build_program(T=T_FULL, TK=1152, rounds=1):
    """Build the per-core Bass program. Same program runs on all 8 cores."""
    assert T % 1024 == 0 and TK % P == 0
    KTT = TK // P  # key tiles
    D = HEAD_DIM
    HT = T // 2  # half of queries (one hf)
    NQC = T // 512  # q chunks
    KC = _chunks(TK, 512)  # xk chunks
    QC = _chunks(T, 512)

    nc = bacc.Bacc("TRN2", target_bir_lowering=False, debug=False, num_devices=1)

    xT = nc.dram_tensor("xT", [D_MODEL, T], BF16, kind="ExternalInput")
    xkT = nc.dram_tensor("xkT", [D_MODEL, TK], BF16, kind="ExternalInput")
    WqT = nc.dram_tensor("WqT", [D_MODEL, HL * D], BF16, kind="ExternalInput")
    WkT = nc.dram_tensor("WkT", [D_MODEL, HL * D], BF16, kind="ExternalInput")
    WvT = nc.dram_tensor("WvT", [D_MODEL, HL * D], BF16, kind="ExternalInput")
    WpT = nc.dram_tensor("WpT", [HL * D, D_MODEL], BF16, kind="ExternalInput")
    kbias = nc.dram_tensor("kbias", [P, KTT], F32, kind="ExternalInput")
    yT = nc.dram_tensor("yT", [D_MODEL, T], BF16, kind="ExternalOutput")

    xT_r = xT.ap().rearrange("(ct p) t -> p ct t", p=P)
    xkT_r = xkT.ap().rearrange("(ct p) t -> p ct t", p=P)
    WqT_r = WqT.ap().rearrange("(ct p) o -> p ct o", p=P)
    WkT_r = WkT.ap().rearrange("(ct p) o -> p ct o", p=P)
    WvT_r = WvT.ap().rearrange("(ct p) o -> p ct o", p=P)
    WpT_r = WpT.ap().rearrange("(ct p) o -> p ct o", p=P)
    yT_r = yT.ap().rearrange("(mt p) t -> p mt t", p=P)

    with tile.TileContext(nc) as tc:
        with (
            tc.tile_pool(name="pers", bufs=1) as pers,
            tc.tile_pool(name="ktp", bufs=2) as ktp,
            tc.tile_pool(name="qtp", bufs=2) as qtp,
            tc.tile_pool(name="attp", bufs=1) as attp,
            tc.tile_pool(name="avsp", bufs=2) as avsp,
            tc.tile_pool(name="xs", bufs=2) as xs,
            tc.tile_pool(name="nrm", bufs=2) as nrm,
            tc.tile_pool(name="drp", bufs=2, space="DRAM") as dp,
            tc.tile_pool(name="ys", bufs=2) as yp,
            tc.tile_pool(name="ps_st", bufs=2, space="PSUM") as ps_st,
            tc.tile_pool(name="ps_av", bufs=1, space="PSUM") as ps_av,
            tc.tile_pool(name="ps_dn", bufs=3, space="PSUM") as ps_dn,
        ):
            # persistent tiles (same addresses every round; WAR via tile deps)
            V_sb = pers.tile([P, KTT, HL, D + 1], BF16, tag="V")
            outT_sb = pers.tile([P, PAIRS, T], BF16, tag="outT")
            kbias_sb = pers.tile([P, KTT], F32, tag="kbias")
            Wq_sb = pers.tile([P, CT, HL * D], BF16, tag="Wq")
            Wk_sb = pers.tile([P, CT, HL * D], BF16, tag="Wk")
            Wv_sb = pers.tile([P, CT, HL * D], BF16, tag="Wv")
            wp_sb = pers.tile([P, PAIRS, D_MODEL], BF16, tag="wp")

            # state keyed by (round, ...) so rounds pipeline into each other
            cur = {}  # (r, "k"/"q", chunk) -> streamed x tile
            KT_t = {}  # (r, pair)
            QT_t = {}  # (r, pair)
            att_t = {}  # (r, pair) -> [hh][hf] tiles
            avs_t = {}  # (r, pair, hh)
            ysb_t = {}  # (r, c)

            # ------------- micro-filler factories: (cost_ns, fn) ------------
            def f_prologue(r):
                def go():
                    nc.sync.dma_start(kbias_sb[:], kbias.ap())
                    nc.sync.dma_start(Wk_sb[:], WkT_r)
                    nc.sync.dma_start(Wq_sb[:], WqT_r)
                    nc.sync.dma_start(Wv_sb[:], WvT_r)

                return [(0, go)]

            def f_xk(r, c):
                s, w = KC[c]

                def go():
                    t = xs.tile([P, CT, 512], BF16, tag="xc", name="xc")
                    nc.sync.dma_start(t[:, :, :w], xkT_r[:, :, s : s + w])
                    cur[(r, "k", c)] = t

                return [(0, go)]

            def f_xq(r, c):
                s, w = QC[c]

                def go():
                    t = xs.tile([P, CT, 512], BF16, tag="xc", name="xc")
                    nc.sync.dma_start(t[:, :, :w], xT_r[:, :, s : s + w])
                    cur[(r, "q", c)] = t

                return [(0, go)]

            def _dense2(r, pair, c, W_sb, dst_t, KTQT):
                """Two micro-ops: 4+4 ct-accumulating matmuls + copy."""
                kind, (s, w) = ("k", KC[c]) if KTQT == "kt" else ("q", QC[c])
                state = {}

                def part(ct0, ct1, final):
                    def go():
                        if "ps" not in state:
                            state["ps"] = ps_dn.tile([P, 512], F32, tag="dn", name="psdn")
                            if (r, pair) not in dst_t:
                                dst_t[(r, pair)] = (
                                    ktp.tile([P, TK], BF16, tag="kt", name=f"kt{pair}")
                                    if KTQT == "kt"
                                    else qtp.tile(
                                        [P, T], BF16, tag="qt", name=f"qt{pair}"
                                    )
                                )
                        ps = state["ps"]
                        xc = cur[(r, kind, c)]
                        for ct in range(ct0, ct1):
                            nc.tensor.matmul(
                                ps[:, :w],
                                lhsT=W_sb[:, ct, pair * P : (pair + 1) * P],
                                rhs=xc[:, ct, :w],
                                start=(ct == 0),
                                stop=(ct == CT - 1),
                            )
                        if final:
                            nc.vector.tensor_copy(
                                out=dst_t[(r, pair)][:, s : s + w], in_=ps[:, :w]
                            )

                    return go

                cost = (4 * w * 5) // 24
                return [(cost, part(0, 4, False)), (cost, part(4, 8, True))]

            def f_kproj(r, pair, c):
                return _dense2(r, pair, c, Wk_sb, KT_t, "kt")

            def f_qproj(r, pair, c):
                return _dense2(r, pair, c, Wq_sb, QT_t, "qt")

            def f_vproj(r, c, tl):
                s, _w = KC[c]
                tt = s // P + tl
                state = {}

                def part(ct0, ct1, final):
                    def go():
                        if "ps" not in state:
                            state["ps"] = ps_dn.tile([P, 512], F32, tag="dn", name="psdn")
                        ps = state["ps"]
                        xc = cur[(r, "k", c)]
                        for ct in range(ct0, ct1):
                            nc.tensor.matmul(
                                ps[:],
                                lhsT=xc[:, ct, tl * P : (tl + 1) * P],
                                rhs=Wv_sb[:, ct, :],
                                start=(ct == 0),
                                stop=(ct == CT - 1),
                            )
                        if final:
                            nc.vector.tensor_copy(
                                out=V_sb[:, tt, :, 0:D],
                                in_=ps[:].rearrange("p (h d) -> p h d", h=HL),
                            )

                    return go

                return [(430, part(0, 4, False)), (430, part(4, 8, True))]

            def f_av(r, pair, hh, c):
                s = c * 512
                hf, so = (0, s) if s < HT else (1, s - HT)
                state = {}

                def part(kt0, kt1, final):
                    def go():
                        if "ps" not in state:
                            state["ps"] = ps_av.tile([P, 512], F32, tag="av", name="psav")
                            if (r, pair, hh) not in avs_t:
                                avs_t[(r, pair, hh)] = avsp.tile(
                                    [D + 1, T], BF16, tag=f"avs{hh}", name=f"avs{hh}"
                                )
                        av = state["ps"]
                        h = 2 * pair + hh
                        at = att_t[(r, pair)][hh][hf]
                        for kt in range(kt0, kt1):
                            nc.tensor.matmul(
                                av[0 : D + 1, :],
                                lhsT=V_sb[:, kt, h, :],
                                rhs=at[:, kt, so : so + 512],
                                start=(kt == 0),
                                stop=(kt == KTT - 1),
                            )
                        if final:
                            nc.vector.tensor_copy(
                                out=avs_t[(r, pair, hh)][:, s : s + 512],
                                in_=av[0 : D + 1, :],
                            )

                    return go

                out = []
                for k0 in range(0, KTT, 3):
                    k1 = min(k0 + 3, KTT)
                    out.append((215 * (k1 - k0), part(k0, k1, k1 == KTT)))
                return out

            def f_norm(r, pair, hh, hf):
                def go():
                    s = hf * HT
                    a = avs_t[(r, pair, hh)]
                    rc = nrm.tile([1, HT], BF16, tag="rc")
                    with nc.allow_low_precision(reason="bf16 softmax norm"):
                        nc.vector.reciprocal(rc[:], a[D : D + 1, s : s + HT])
                    rcd = dp.tile([1, HT], BF16, tag="rcd")
                    nc.sync.dma_start(rcd[:], rc[:])
                    rcb = nrm.tile([D, HT], BF16, tag="rcb")
                    nc.sync.dma_start(rcb[:], rcd[0:1, :].to_broadcast((D, HT)))
                    nc.vector.tensor_mul(
                        out=outT_sb[hh * 64 : (hh + 1) * 64, pair, s : s + HT],
                        in0=a[0:D, s : s + HT],
                        in1=rcb[:],
                    )

                return [(0, go)]

            def f_wpdma():
                def go():
                    nc.sync.dma_start(wp_sb[:], WpT_r)

                return [(0, go)]

            def f_proj(r, m, c, ptag):
                s = c * 512

                def go():
                    if (r, c) not in ysb_t:
                        ysb_t[(r, c)] = yp.tile(
                            [P, CT, 512], BF16, tag="ysb", name="ysb"
                        )
                    pool_ = ps_st if ptag == "st" else ps_dn
                    shape = [P, HT] if ptag == "st" else [P, 512]
                    ps = pool_.tile(shape, F32, tag=ptag)
                    for ctp in range(PAIRS):
                        nc.tensor.matmul(
                            ps[:, :512],
                            lhsT=wp_sb[:, ctp, m * P : (m + 1) * P],
                            rhs=outT_sb[:, ctp, s : s + 512],
                            start=(ctp == 0),
                            stop=(ctp == PAIRS - 1),
                        )
                    nc.vector.tensor_copy(out=ysb_t[(r, c)][:, m, :], in_=ps[:, :512])

                return [(440, go)]

            def f_ydma(r, c):
                s = c * 512

                def go():
                    nc.sync.dma_start(yT_r[:, :, s : s + 512], ysb_t[(r, c)][:])
                    del ysb_t[(r, c)]

                return [(0, go)]

            def ramp_items(r):
                """Everything the first window of round r needs."""
                out = f_prologue(r)
                for c in range(len(KC)):
                    out += f_xk(r, c) + f_kproj(r, 0, c)
                for c in (0, 1):
                    out += f_xq(r, c) + f_qproj(r, 0, c)
                return out

            def tail_av_items(r):
                """attn@V + normalize for round r's last pair, hf=1 half."""
                last = PAIRS - 1
                out = []
                for c in (NQC // 2, NQC // 2 + 1):
                    out += f_av(r, last, 0, c)
                    out += f_av(r, last, 1, c)
                out += f_norm(r, last, 0, 1)
                out += f_norm(r, last, 1, 1)
                return out

            def tail_proj_items(r, use_st=False):
                """Output projection for round r's second-half query chunks."""
                out = []
                for c in (2, 3):
                    for m in range(CT):
                        ptag = ("dn", "st", "st")[m % 3] if use_st else "dn"
                        out += f_proj(r, m, c, ptag)
                    out += f_ydma(r, c)
                return out

            # ------------- score/exp backbone -------------------------------
            TILE_BUDGET = 800  # ns of filler PE work per score tile

            def emit_half_sts(r, pair, hf, fillers):
                idx = 0
                credit = 0
                ntiles = 2 * KTT
                tot = sum(cst for cst, _ in fillers)
                per_tile = max(TILE_BUDGET, (tot + ntiles - 1) // ntiles)
                for ti in range(ntiles):
                    kt, hh = divmod(ti, 2)
                    base = hh * 64
                    pst = ps_st.tile([P, HT], F32, tag="st")
                    for s2 in (0, 512):
                        nc.tensor.matmul(
                            pst[:, s2 : s2 + 512],
                            lhsT=KT_t[(r, pair)][
                                base : base + 64, kt * P : (kt + 1) * P
                            ],
                            rhs=QT_t[(r, pair)][
                                base : base + 64, hf * HT + s2 : hf * HT + s2 + 512
                            ],
                            start=True,
                            stop=True,
                        )
                    nc.scalar.activation(
                        att_t[(r, pair)][hh][hf][:, kt, :],
                        pst[:],
                        mybir.ActivationFunctionType.Exp,
                        bias=kbias_sb[:, kt : kt + 1],
                        scale=0.125,
                    )
                    credit += per_tile
                    while idx < len(fillers) and fillers[idx][0] <= credit:
                        credit -= fillers[idx][0]
                        fillers[idx][1]()
                        idx += 1
                while idx < len(fillers):
                    fillers[idx][1]()
                    idx += 1

            def alloc_att(r, pair):
                att_t[(r, pair)] = [
                    [
                        attp.tile(
                            [P, KTT, HT], BF16, tag=f"at{hh}{hf}", name=f"at{hh}{hf}"
                        )
                        for hf in (0, 1)
                    ]
                    for hh in (0, 1)
                ]

            # ones column of V is constant across rounds
            nc.gpsimd.memset(V_sb[:, :, :, D : D + 1], 1.0)

            # ------------- pipelined rounds x pairs -------------------------
            for _cst, fn in ramp_items(0):
                fn()

            for r in range(rounds):
                for pair in range(PAIRS):
                    alloc_att(r, pair)
                    fa = []  # fillers during hf=0 tiles
                    fb = []  # fillers during hf=1 tiles

                    if pair == 0:
                        if r > 0:
                            fa += tail_av_items(r - 1)
                        for c in range(len(KC)):
                            fa += f_xk(r, c) + f_kproj(r, 1, c)
                            for tl in range(KC[c][1] // P):
                                fa += f_vproj(r, c, tl)
                        for c in (2, 3):
                            fa += f_xq(r, c) + f_qproj(r, 0, c)
                        if r > 0:
                            fb += tail_proj_items(r - 1)
                        for c in range(NQC):
                            fb += f_xq(r, c) + f_qproj(r, 1, c)
                    else:
                        for c in (NQC // 2, NQC // 2 + 1):
                            fa += f_av(r, pair - 1, 0, c)
                            fa += f_av(r, pair - 1, 1, c)
                        fa += f_norm(r, pair - 1, 0, 1)
                        fa += f_norm(r, pair - 1, 1, 1)
                        if pair < PAIRS - 1:
                            for c in range(len(KC)):
                                fa += f_xk(r, c) + f_kproj(r, pair + 1, c)
                            for c in range(NQC):
                                fa += f_xq(r, c) + f_qproj(r, pair + 1, c)
                        else:
                            fa += f_wpdma()
                            if r + 1 < rounds:
                                fa += ramp_items(r + 1)

                    # this pair's hf=0 AV + norm (strictly after hf=0 tiles)
                    for c in (0, 1):
                        fb += f_av(r, pair, 0, c)
                        fb += f_av(r, pair, 1, c)
                    fb += f_norm(r, pair, 0, 0)
                    fb += f_norm(r, pair, 1, 0)
                    if pair == PAIRS - 1:
                        for c in (0, 1):
                            for m in range(CT):
                                fb += f_proj(r, m, c, "dn")
                            fb += f_ydma(r, c)

                    emit_half_sts(r, pair, 0, fa)
                    emit_half_sts(r, pair, 1, fb)

            for _cst, fn in tail_av_items(rounds - 1):
                fn()
            for _cst, fn in tail_proj_items(rounds - 1, use_st=True):
                fn()

    return nc


def prep_core_inputs(x, pad_mask, W_qkv, W_proj, b, hg, TK):
    """Host-side shard prep for core (b, hg)."""
    C = D_MODEL
    D = HEAD_DIM
    xb = np.asarray(x[b], dtype=np.float32)  # [T, C]
    mask = np.asarray(pad_mask[b])
    idx = np.nonzero(~mask)[0]
    cnt = len(idx)
    assert cnt <= TK, f"key count {cnt} exceeds TK={TK}"

    BF = ml_dtypes.bfloat16
    xT = np.ascontiguousarray(xb.T).astype(BF)  # [C, T]
    xkT = np.zeros((C, TK), dtype=BF)
    xkT[:, :cnt] = xb[idx].T.astype(BF)

    kb = np.zeros((TK,), dtype=np.float32)
    kb[cnt:] = -30000.0
    kbias = np.ascontiguousarray(kb.reshape(TK // P, P).T)  # [128, KTT]

    Wq = W_qkv[0:C].reshape(N_HEADS, D, C)
    Wk = W_qkv[C : 2 * C].reshape(N_HEADS, D, C)
    Wv = W_qkv[2 * C : 3 * C].reshape(N_HEADS, D, C)
    heads = range(hg * HL, (hg + 1) * HL)
    WqT = np.ascontiguousarray(np.concatenate([Wq[h] for h in heads], axis=0).T)
    WkT = np.ascontiguousarray(np.concatenate([Wk[h] for h in heads], axis=0).T)
    WvT = np.ascontiguousarray(np.concatenate([Wv[h] for h in heads], axis=0).T)
    WpT = np.ascontiguousarray(
        np.concatenate([W_proj[:, h * D : (h + 1) * D] for h in heads], axis=1).T
    )
    return {
        "xT": xT,
        "xkT": xkT,
        "WqT": WqT.astype(BF),
        "WkT": WkT.astype(BF),
        "WvT": WvT.astype(BF),
        "WpT": WpT.astype(BF),
        "kbias": kbias,
    }


def kernel(x, pad_mask, W_qkv, W_proj):
    x = np.asarray(x, dtype=np.float32)
    pad_mask = np.asarray(pad_mask, dtype=bool)
    W_qkv = np.asarray(W_qkv, dtype=np.float32)
    W_proj = np.asarray(W_proj, dtype=np.float32)
    Bv, T, C = x.shape

    counts = (~pad_mask).sum(axis=1)
    TK = max(int(-(-counts.max() // P)) * P, P)

    nc = build_program(T=T, TK=TK)
    nc.compile()
    nc.m = get_hw_module(nc.m)

    in_maps = []
    for c in range(8):
        b, hg = c // 2, c % 2
        in_maps.append(prep_core_inputs(x, pad_mask, W_qkv, W_proj, b, hg, TK))

    res = None
    for attempt in range(3):
        try:
            res = run_bass_kernel_spmd(nc, in_maps, core_ids=list(range(8)))
            break
        except Exception:
            if attempt == 2:
                raise
            import time as _time

            _time.sleep(5.0)

    y = np.empty((Bv, T, C), dtype=np.float32)
    for b in range(Bv):
        yT = res.results[2 * b]["yT"].astype(np.float32) + res.results[2 * b + 1][
            "yT"
        ].astype(np.float32)
        y[b] = yT.T
    return y


# revision 15
# speedup vs baseline: 1.2239x; 1.2170x over previous
# Bidirectional multi-head attention (key-padding mask) on 8 Trainium2 cores.
#
# Sharding: core = (batch b, head-group hg); B=4 x 2 head-groups of 8 heads.
# Each core computes y_partial^T [C, T] for its 8 heads of batch b; host sums
# the two head-group partials per batch and transposes back to [T, C].
#
# Masked keys are gathered away on the host (exactly equivalent to the -inf
# mask), padded to a multiple of 128 with bias -30000 (exp underflows to 0).
#
# Schedule: software-pipelined over head pairs (and across timing rounds).
# The scores+exp stream is the ACT-paced backbone; all other PE work (QKV
# projections for the next pair, V projection, attn@V for the previous pair,
# output projection, next round's ramp) is interleaved into the PE stream as
# micro-fillers (~400-900ns of PE work each) so the PE neither starves the
# ACT engine nor idles. Score matmuls for the two heads of a pair sit on
# partitions 0-63 / 64-127, so adjacent score matmuls target disjoint PE row
# groups and overlap on hardware.
#
# Device layouts (partition dim first):
#   KT(p)   [128, TK]   k^T for pair p (head 2p on parts 0-63, 2p+1 on 64-127)
#   QT(p)   [128, T]    q^T likewise
#   V_sb    [128, KTT, 8, 65] bf16: V rows (key on partitions) + ones col
#   S^T     = matmul(lhsT=KT slice [64,128], rhs=QT [64,512]) -> [128 keys, q]
#   exp     fused on ScalarE: exp(s/8 + bias_k), bias per key partition
#   attn@V  = matmul(lhsT=V_aug [128,65], rhs=attnT [128,512]) accum over kt
#             row 64 of the psum = per-query row-sum (ones column trick)
#   norm    reciprocal(rowsum), DRAM-roundtrip broadcast to 64 partitions,
#             then DVE multiply into outT
#   proj    y^T = matmul(lhsT=WpT strip, rhs=outT) accum over 4 pairs

import sys

import ml_dtypes
import numpy as np

try:
    import concourse.bacc as bacc  # noqa: F401
except ImportError:
    sys.path.insert(0, "/opt/trn_rl_repo")

import concourse.bacc as bacc
import concourse.bass as bass
import concourse.mybir as mybir
import concourse.tile as tile
from concourse.bass_interp import get_hw_module
from concourse.bass_utils import run_bass_kernel_spmd

F32 = mybir.dt.float32
BF16 = mybir.dt.bfloat16
P = 128

D_MODEL = 1024
N_HEADS = 16
HEAD_DIM = 64
B = 4
T_FULL = 2048
HL = 8  # heads per core
PAIRS = HL // 2
CT = D_MODEL // P  # c tiles


def _chunks(total, size):
    out = []
    s = 0
    while s < total:
        out.append((s, min(size, total - s)))
        s += size
    return out


def build_program(T=T_FULL, TK=1152, rounds=1, ablate=None):
    """Build the per-core Bass program. Same program runs on all 8 cores.

    ablate: None | "noexp" | "noav" | "nodense" — timing diagnostics only.
    """
    assert T % 1024 == 0 and TK % P == 0
    KTT = TK // P  # key tiles
    D = HEAD_DIM
    HT = T // 2  # half of queries (one hf)
    NQC = T // 512  # q chunks
    KC = _chunks(TK, 512)  # xk chunks
    QC = _chunks(T, 512)

    nc = bacc.Bacc("TRN2", target_bir_lowering=False, debug=False, num_devices=1)

    xT = nc.dram_tensor("xT", [D_MODEL, T], BF16, kind="ExternalInput")
    xkT = nc.dram_tensor("xkT", [D_MODEL, TK], BF16, kind="ExternalInput")
    WqT = nc.dram_tensor("WqT", [D_MODEL, HL * D], BF16, kind="ExternalInput")
    WkT = nc.dram_tensor("WkT", [D_MODEL, HL * D], BF16, kind="ExternalInput")
    WvT = nc.dram_tensor("WvT", [D_MODEL, HL * D], BF16, kind="ExternalInput")
    WpT = nc.dram_tensor("WpT", [HL * D, D_MODEL], BF16, kind="ExternalInput")
    kbias = nc.dram_tensor("kbias", [P, KTT], F32, kind="ExternalInput")
    yT = nc.dram_tensor("yT", [D_MODEL, T], BF16, kind="ExternalOutput")

    xT_r = xT.ap().rearrange("(ct p) t -> p ct t", p=P)
    xkT_r = xkT.ap().rearrange("(ct p) t -> p ct t", p=P)
    WqT_r = WqT.ap().rearrange("(ct p) o -> p ct o", p=P)
    WkT_r = WkT.ap().rearrange("(ct p) o -> p ct o", p=P)
    WvT_r = WvT.ap().rearrange("(ct p) o -> p ct o", p=P)
    WpT_r = WpT.ap().rearrange("(ct p) o -> p ct o", p=P)
    yT_r = yT.ap().rearrange("(mt p) t -> p mt t", p=P)

    with tile.TileContext(nc) as tc:
        with (
            tc.tile_pool(name="pers", bufs=1) as pers,
            tc.tile_pool(name="ktp", bufs=2) as ktp,
            tc.tile_pool(name="qtp", bufs=2) as qtp,
            tc.tile_pool(name="attp", bufs=1) as attp,
            tc.tile_pool(name="avsp", bufs=2) as avsp,
            tc.tile_pool(name="xs", bufs=2) as xs,
            tc.tile_pool(name="nrm", bufs=2) as nrm,
            tc.tile_pool(name="drp", bufs=2, space="DRAM") as dp,
            tc.tile_pool(name="ys", bufs=2) as yp,
            tc.tile_pool(name="ps_st", bufs=2, space="PSUM") as ps_st,
            tc.tile_pool(name="ps_av", bufs=1, space="PSUM") as ps_av,
            tc.tile_pool(name="ps_dn", bufs=3, space="PSUM") as ps_dn,
        ):
            # persistent tiles (same addresses every round; WAR via tile deps)
            V_sb = pers.tile([P, KTT, HL, D + 1], BF16, tag="V")
            outT_sb = pers.tile([P, PAIRS, T], BF16, tag="outT")
            kbias_sb = pers.tile([P, KTT], F32, tag="kbias")
            Wq_sb = pers.tile([P, CT, HL * D], BF16, tag="Wq")
            Wk_sb = pers.tile([P, CT, HL * D], BF16, tag="Wk")
            Wv_sb = pers.tile([P, CT, HL * D], BF16, tag="Wv")
            wp_sb = pers.tile([P, PAIRS, D_MODEL], BF16, tag="wp")
            ones64 = pers.tile([1, HEAD_DIM], BF16, tag="ones64")

            # state keyed by (round, ...) so rounds pipeline into each other
            cur = {}  # (r, "k"/"q", chunk) -> streamed x tile
            KT_t = {}  # (r, pair)
            QT_t = {}  # (r, pair)
            att_t = {}  # (r, pair) -> [hh][hf] tiles
            avs_t = {}  # (r, pair, hh)
            ysb_t = {}  # (r, c)

            # ------------- micro-filler factories: (cost_ns, fn) ------------
            def f_prologue(r):
                def go():
                    nc.sync.dma_start(kbias_sb[:], kbias.ap())
                    nc.sync.dma_start(Wk_sb[:], WkT_r)
                    nc.sync.dma_start(Wq_sb[:], WqT_r)
                    nc.sync.dma_start(Wv_sb[:], WvT_r)

                return [(0, go)]

            def f_xk(r, c):
                s, w = KC[c]

                def go():
                    t = xs.tile([P, CT, 512], BF16, tag="xc", name="xc")
                    nc.sync.dma_start(t[:, :, :w], xkT_r[:, :, s : s + w])
                    cur[(r, "k", c)] = t

                return [(0, go)]

            def f_xq(r, c):
                s, w = QC[c]

                def go():
                    t = xs.tile([P, CT, 512], BF16, tag="xc", name="xc")
                    nc.sync.dma_start(t[:, :, :w], xT_r[:, :, s : s + w])
                    cur[(r, "q", c)] = t

                return [(0, go)]

            def _dense2(r, pair, c, W_sb, dst_t, KTQT):
                """Two micro-ops: 4+4 ct-accumulating matmuls + copy."""
                kind, (s, w) = ("k", KC[c]) if KTQT == "kt" else ("q", QC[c])
                state = {}

                def part(ct0, ct1, final):
                    def go():
                        if "ps" not in state:
                            state["ps"] = ps_dn.tile([P, 512], F32, tag="dn", name="psdn")
                            if (r, pair) not in dst_t:
                                dst_t[(r, pair)] = (
                                    ktp.tile([P, TK], BF16, tag="kt", name=f"kt{pair}")
                                    if KTQT == "kt"
                                    else qtp.tile(
                                        [P, T], BF16, tag="qt", name=f"qt{pair}"
                                    )
                                )
                        ps = state["ps"]
                        xc = cur[(r, kind, c)]
                        for ct in range(ct0, ct1):
                            nc.tensor.matmul(
                                ps[:, :w],
                                lhsT=W_sb[:, ct, pair * P : (pair + 1) * P],
                                rhs=xc[:, ct, :w],
                                start=(ct == 0),
                                stop=(ct == CT - 1),
                            )
                        if final:
                            nc.vector.tensor_copy(
                                out=dst_t[(r, pair)][:, s : s + w], in_=ps[:, :w]
                            )

                    return go

                cost = (4 * w * 5) // 24
                return [(cost, part(0, 4, False)), (cost, part(4, 8, True))]

            def f_kproj(r, pair, c):
                return _dense2(r, pair, c, Wk_sb, KT_t, "kt")

            def f_qproj(r, pair, c):
                return _dense2(r, pair, c, Wq_sb, QT_t, "qt")

            def f_vproj(r, c, tl):
                s, _w = KC[c]
                tt = s // P + tl
                state = {}

                def part(ct0, ct1, final):
                    def go():
                        if "ps" not in state:
                            state["ps"] = ps_dn.tile([P, 512], F32, tag="dn", name="psdn")
                        ps = state["ps"]
                        xc = cur[(r, "k", c)]
                        for ct in range(ct0, ct1):
                            nc.tensor.matmul(
                                ps[:],
                                lhsT=xc[:, ct, tl * P : (tl + 1) * P],
                                rhs=Wv_sb[:, ct, :],
                                start=(ct == 0),
                                stop=(ct == CT - 1),
                            )
                        if final:
                            nc.vector.tensor_copy(
                                out=V_sb[:, tt, :, 0:D],
                                in_=ps[:].rearrange("p (h d) -> p h d", h=HL),
                            )

                    return go

                return [(430, part(0, 4, False)), (430, part(4, 8, True))]

            def f_av(r, pair, hh, c):
                if ablate == "noav":
                    return []
                s = c * 512
                hf, so = (0, s) if s < HT else (1, s - HT)
                state = {}

                def part(kt0, kt1, final):
                    def go():
                        if "ps" not in state:
                            state["ps"] = ps_av.tile([P, 512], F32, tag="av", name="psav")
                            if (r, pair, hh) not in avs_t:
                                avs_t[(r, pair, hh)] = avsp.tile(
                                    [D + 1, T], BF16, tag=f"avs{hh}", name=f"avs{hh}"
                                )
                        av = state["ps"]
                        h = 2 * pair + hh
                        at = att_t[(r, pair)][hh][hf]
                        for kt in range(kt0, kt1):
                            nc.tensor.matmul(
                                av[0 : D + 1, :],
                                lhsT=V_sb[:, kt, h, :],
                                rhs=at[:, kt, so : so + 512],
                                start=(kt == 0),
                                stop=(kt == KTT - 1),
                            )
                        if final:
                            nc.vector.tensor_copy(
                                out=avs_t[(r, pair, hh)][:, s : s + 512],
                                in_=av[0 : D + 1, :],
                            )

                    return go

                out = []
                for k0 in range(0, KTT, 3):
                    k1 = min(k0 + 3, KTT)
                    out.append((215 * (k1 - k0), part(k0, k1, k1 == KTT)))
                return out

            def f_norm(r, pair, hh, hf):
                if ablate == "noav":
                    return []
                state = {}

                def recip():
                    s = hf * HT
                    a = avs_t[(r, pair, hh)]
                    rc = nrm.tile([1, HT], BF16, tag="rc")
                    state["rc"] = rc
                    with nc.allow_low_precision(reason="bf16 softmax norm"):
                        nc.vector.reciprocal(rc[:], a[D : D + 1, s : s + HT])

                def mulc(ci):
                    def go():
                        s = hf * HT + ci * 512
                        a = avs_t[(r, pair, hh)]
                        rc = state["rc"]
                        # broadcast 1/rowsum across 64 partitions on the PE:
                        # [64,512] = ones64^T @ rc_chunk  (K=1 matmul)
                        ps = ps_dn.tile([P, 512], F32, tag="dn", name="psdn")
                        nc.tensor.matmul(
                            ps[0:D, :],
                            lhsT=ones64[:],
                            rhs=rc[0:1, ci * 512 : ci * 512 + 512],
                            start=True,
                            stop=True,
                        )
                        nc.vector.tensor_mul(
                            out=outT_sb[hh * 64 : (hh + 1) * 64, pair, s : s + 512],
                            in0=a[0:D, s : s + 512],
                            in1=ps[0:D, :],
                        )

                    return go

                return [(0, recip)] + [(250, mulc(ci)) for ci in range(HT // 512)]

            def f_wpdma():
                def go():
                    nc.sync.dma_start(wp_sb[:], WpT_r)

                return [(0, go)]

            def f_proj(r, m, c, ptag):
                if ablate == "noav":
                    return []
                s = c * 512

                def go():
                    if (r, c) not in ysb_t:
                        ysb_t[(r, c)] = yp.tile(
                            [P, CT, 512], BF16, tag="ysb", name="ysb"
                        )
                    pool_ = ps_st if ptag == "st" else ps_dn
                    shape = [P, HT] if ptag == "st" else [P, 512]
                    ps = pool_.tile(shape, F32, tag=ptag)
                    for ctp in range(PAIRS):
                        nc.tensor.matmul(
                            ps[:, :512],
                            lhsT=wp_sb[:, ctp, m * P : (m + 1) * P],
                            rhs=outT_sb[:, ctp, s : s + 512],
                            start=(ctp == 0),
                            stop=(ctp == PAIRS - 1),
                        )
                    nc.vector.tensor_copy(out=ysb_t[(r, c)][:, m, :], in_=ps[:, :512])

                return [(440, go)]

            def f_ydma(r, c):
                if ablate == "noav":
                    return []
                s = c * 512

                def go():
                    nc.sync.dma_start(yT_r[:, :, s : s + 512], ysb_t[(r, c)][:])
                    del ysb_t[(r, c)]

                return [(0, go)]

            def k_stream_section(r, kpair, with_v):
                items = list(f_xk(r, 0))
                n = len(KC)
                for c in range(n):
                    if c + 1 < n:
                        items += f_xk(r, c + 1)
                    items += f_kproj(r, kpair, c)
                    if with_v:
                        for tl in range(KC[c][1] // P):
                            items += f_vproj(r, c, tl)
                return items

            def q_stream_section(r, qpair, cs=None):
                cs = list(range(NQC)) if cs is None else cs
                items = list(f_xq(r, cs[0]))
                for i, c in enumerate(cs):
                    if i + 1 < len(cs):
                        items += f_xq(r, cs[i + 1])
                    items += f_qproj(r, qpair, c)
                return items

            def ramp_items(r):
                """Everything the first window of round r needs."""
                out = f_prologue(r)
                out += k_stream_section(r, 0, with_v=False)
                out += q_stream_section(r, 0, cs=[0, 1])
                return out

            def tail_av_items(r):
                """attn@V + normalize for round r's last pair, hf=1 half."""
                last = PAIRS - 1
                out = []
                for c in (NQC // 2, NQC // 2 + 1):
                    out += f_av(r, last, 0, c)
                    out += f_av(r, last, 1, c)
                out += f_norm(r, last, 0, 1)
                out += f_norm(r, last, 1, 1)
                return out

            def tail_proj_items(r, use_st=False):
                """Output projection for round r's second-half query chunks."""
                out = []
                for c in (2, 3):
                    for m in range(CT):
                        ptag = ("dn", "st", "st")[m % 3] if use_st else "dn"
                        out += f_proj(r, m, c, ptag)
                    out += f_ydma(r, c)
                return out

            # ------------- score/exp backbone -------------------------------
            TILE_BUDGET = 800  # ns of filler PE work per score tile

            def emit_half_sts(r, pair, hf, fillers):
                idx = 0
                credit = 0
                ntiles = 2 * KTT
                tot = sum(cst for cst, _ in fillers)
                per_tile = max(TILE_BUDGET, (tot + ntiles - 1) // ntiles)
                for ti in range(ntiles):
                    kt, hh = divmod(ti, 2)
                    base = hh * 64
                    pst = ps_st.tile([P, HT], F32, tag="st")
                    for s2 in (0, 512):
                        nc.tensor.matmul(
                            pst[:, s2 : s2 + 512],
                            lhsT=KT_t[(r, pair)][
                                base : base + 64, kt * P : (kt + 1) * P
                            ],
                            rhs=QT_t[(r, pair)][
                                base : base + 64, hf * HT + s2 : hf * HT + s2 + 512
                            ],
                            start=True,
                            stop=True,
                        )
                    if ablate != "noexp":
                        nc.scalar.activation(
                            att_t[(r, pair)][hh][hf][:, kt, :],
                            pst[:],
                            mybir.ActivationFunctionType.Exp,
                            bias=kbias_sb[:, kt : kt + 1],
                            scale=0.125,
                        )
                    credit += per_tile
                    while idx < len(fillers) and fillers[idx][0] <= credit:
                        credit -= fillers[idx][0]
                        fillers[idx][1]()
                        idx += 1
                while idx < len(fillers):
                    fillers[idx][1]()
                    idx += 1

            def alloc_att(r, pair):
                att_t[(r, pair)] = [
                    [
                        attp.tile(
                            [P, KTT, HT], BF16, tag=f"at{hh}{hf}", name=f"at{hh}{hf}"
                        )
                        for hf in (0, 1)
                    ]
                    for hh in (0, 1)
                ]

            # ones column of V / broadcast row are constant across rounds
            nc.gpsimd.memset(V_sb[:, :, :, D : D + 1], 1.0)
            nc.gpsimd.memset(ones64[:], 1.0)

            # ------------- pipelined rounds x pairs -------------------------
            for _cst, fn in ramp_items(0):
                fn()

            for r in range(rounds):
                for pair in range(PAIRS):
                    alloc_att(r, pair)
                    fa = []  # fillers during hf=0 tiles
                    fb = []  # fillers during hf=1 tiles

                    if pair == 0:
                        if r > 0:
                            fa += tail_av_items(r - 1)
                        fa += k_stream_section(r, 1, with_v=True)
                        fa += q_stream_section(r, 0, cs=[2, 3])
                        if r > 0:
                            fb += tail_proj_items(r - 1)
                        fb += q_stream_section(r, 1)
                    else:
                        for c in (NQC // 2, NQC // 2 + 1):
                            fa += f_av(r, pair - 1, 0, c)
                            fa += f_av(r, pair - 1, 1, c)
                        fa += f_norm(r, pair - 1, 0, 1)
                        fa += f_norm(r, pair - 1, 1, 1)
                        if pair < PAIRS - 1:
                            fa += k_stream_section(r, pair + 1, with_v=False)
                            fa += q_stream_section(r, pair + 1)
                        else:
                            fa += f_wpdma()
                            if r + 1 < rounds:
                                fa += ramp_items(r + 1)

                    # this pair's hf=0 AV + norm (strictly after hf=0 tiles)
                    for c in (0, 1):
                        fb += f_av(r, pair, 0, c)
                        fb += f_av(r, pair, 1, c)
                    fb += f_norm(r, pair, 0, 0)
                    fb += f_norm(r, pair, 1, 0)
                    if pair == PAIRS - 1:
                        for c in (0, 1):
                            for m in range(CT):
                                fb += f_proj(r, m, c, "dn")
                            fb += f_ydma(r, c)

                    emit_half_sts(r, pair, 0, fa)
                    emit_half_sts(r, pair, 1, fb)

            for _cst, fn in tail_av_items(rounds - 1):
                fn()
            for _cst, fn in tail_proj_items(rounds - 1, use_st=True):
                fn()

    return nc


def prep_core_inputs(x, pad_mask, W_qkv, W_proj, b, hg, TK):
    """Host-side shard prep for core (b, hg)."""
    C = D_MODEL
    D = HEAD_DIM
    xb = np.asarray(x[b], dtype=np.float32)  # [T, C]
    mask = np.asarray(pad_mask[b])
    idx = np.nonzero(~mask)[0]
    cnt = len(idx)
    assert cnt <= TK, f"key count {cnt} exceeds TK={TK}"

    BF = ml_dtypes.bfloat16
    xT = np.ascontiguousarray(xb.T).astype(BF)  # [C, T]
    xkT = np.zeros((C, TK), dtype=BF)
    xkT[:, :cnt] = xb[idx].T.astype(BF)

    kb = np.zeros((TK,), dtype=np.float32)
    kb[cnt:] = -30000.0
    kbias = np.ascontiguousarray(kb.reshape(TK // P, P).T)  # [128, KTT]

    Wq = W_qkv[0:C].reshape(N_HEADS, D, C)
    Wk = W_qkv[C : 2 * C].reshape(N_HEADS, D, C)
    Wv = W_qkv[2 * C : 3 * C].reshape(N_HEADS, D, C)
    heads = range(hg * HL, (hg + 1) * HL)
    WqT = np.ascontiguousarray(np.concatenate([Wq[h] for h in heads], axis=0).T)
    WkT = np.ascontiguousarray(np.concatenate([Wk[h] for h in heads], axis=0).T)
    WvT = np.ascontiguousarray(np.concatenate([Wv[h] for h in heads], axis=0).T)
    WpT = np.ascontiguousarray(
        np.concatenate([W_proj[:, h * D : (h + 1) * D] for h in heads], axis=1).T
    )
    return {
        "xT": xT,
        "xkT": xkT,
        "WqT": WqT.astype(BF),
        "WkT": WkT.astype(BF),
        "WvT": WvT.astype(BF),
        "WpT": WpT.astype(BF),
        "kbias": kbias,
    }


def kernel(x, pad_mask, W_qkv, W_proj):
    x = np.asarray(x, dtype=np.float32)
    pad_mask = np.asarray(pad_mask, dtype=bool)
    W_qkv = np.asarray(W_qkv, dtype=np.float32)
    W_proj = np.asarray(W_proj, dtype=np.float32)
    Bv, T, C = x.shape

    counts = (~pad_mask).sum(axis=1)
    TK = max(int(-(-counts.max() // P)) * P, P)

    nc = build_program(T=T, TK=TK)
    nc.compile()
    nc.m = get_hw_module(nc.m)

    in_maps = []
    for c in range(8):
        b, hg = c // 2, c % 2
        in_maps.append(prep_core_inputs(x, pad_mask, W_qkv, W_proj, b, hg, TK))

    res = None
    for attempt in range(3):
        try:
            res = run_bass_kernel_spmd(nc, in_maps, core_ids=list(range(8)))
            break
        except Exception:
            if attempt == 2:
                raise
            import time as _time

            _time.sleep(5.0)

    y = np.empty((Bv, T, C), dtype=np.float32)
    for b in range(Bv):
        yT = res.results[2 * b]["yT"].astype(np.float32) + res.results[2 * b + 1][
            "yT"
        ].astype(np.float32)
        y[b] = yT.T
    return y
